# revision 13
# baseline (speedup 1.0000x reference)
"""Holt-Winters exponential smoothing (level/trend/seasonal, P=7) on 8 Trainium2
NeuronCores.

Math: the per-row recurrence is linear in a 9-dim state
s = [level, trend, buf_0..buf_6]:  s_t = A_{t%7} s_{t-1} + c_{t%7} x_t.
Steps t=1..4095 are processed in 117 chunks of C=35 steps (35 % 7 == 0 so every
chunk sees the same slot pattern and shares one coefficient set), grouped into
9 groups of G=13 chunks.  Per chunk the outputs are a matmul
  Y_c (105,B) = [Wm | U]^T @ [X_c (35,B); sigma_c (9,B)]
and the chunk-entry states sigma_c come from a per-group prefix-scan matmul
over the group's stacked inputs.  All heavy compute runs on the TensorEngine;
the only sequential dependency is the 9-link group chain.

Precision/speed: inputs and coefficients are split hi/lo into bf16 pairs
(x = hi + lo exactly to ~2^-18 relative).  A full-precision product needs
three bf16 matmuls (Wh.Xh, Wl.Xh, Wh.Xl); the two cross terms are fused into
one K-stacked matmul over [X_hi; X_lo], so each logical fp32 matmul costs two
bf16 matmuls (1 cycle/row each vs fp32's 4).  All products are exact in fp32
PSUM accumulation; the dropped Wl.Xl term is ~2^-18 relative.

Sharding: pure data-parallel over the batch axis (1024 rows per core).
"""

import numpy as np

P = 7
C = 35            # chunk size (steps); 35 % 7 == 0
G = 13            # chunks per group
NG = 9            # groups; NG*G*C == L-1
L = 4096
B = 8192
NCORES = 8
BL = B // NCORES  # 1024 batch rows per core
NHALF = 512       # matmul moving-dim tile (fp32 PSUM bank limit)


def _sigmoid(z):
    return 1.0 / (1.0 + np.exp(-z))


def _step_mats(a, b, g):
    """A_i (9x9), c_i (9,) for seasonal slot i, float64."""
    A, c = [], []
    for i in range(P):
        col = 2 + i
        Ai = np.zeros((9, 9), np.float64)
        ci = np.zeros(9, np.float64)
        Ai[0, 0] = 1 - a
        Ai[0, 1] = 1 - a
        Ai[0, col] += -a
        Ai[1, 0] = -a * b
        Ai[1, 1] = 1 - a * b
        Ai[1, col] += -a * b
        for j in range(P):
            Ai[2 + j, 2 + j] = 1.0
        Ai[col, :] = 0.0
        Ai[col, 0] = -g * (1 - a)
        Ai[col, 1] = -g * (1 - a)
        Ai[col, col] = g * a + 1 - g
        ci[0] = a
        ci[1] = a * b
        ci[col] = g * (1 - a)
        A.append(Ai)
        c.append(ci)
    return A, c


def _hi_lo(x):
    import ml_dtypes
    hi = x.astype(np.float32).astype(ml_dtypes.bfloat16)
    lo = (x.astype(np.float32) - hi.astype(np.float32)).astype(ml_dtypes.bfloat16)
    return hi, lo


def _build_coeffs(alpha, beta, gamma):
    """Host-precomputed stationary matrices (float64 -> bf16 hi/lo splits).

    lhsT layouts (K x M):
      wm1 (79, 105):  [Wm_hi; 0_{35}; U_hi]          (hi.hi pass)
      wm2 (88, 105):  [Wm_lo; Wm_hi; U_lo; U_hi]     (cross pass over [Xh;Xl;sh;sl])
      ws1h/ws1l (126, 126): scan state-propagation lhsT, hi and lo
      wqa (13, 35, 126):  per-chunk scan lhsT, hi
      wqb (13, 70, 126):  per-chunk scan lhsT cross [wqv_lo; wqv_hi]
      winit (7, 126) f32: init matmul (y_0 rows 0..2, s_0 rows 117..125)
    """
    import ml_dtypes
    a, b, g = _sigmoid(alpha), _sigmoid(beta), _sigmoid(gamma)
    A, c = _step_mats(a, b, g)
    slots = [(1 + k) % P for k in range(C)]

    Phi = np.zeros((C, 9, 9), np.float64)
    w = np.zeros((C, C, 9), np.float64)
    cur = np.eye(9)
    for k in range(C):
        i = slots[k]
        if k > 0:
            w[k, :k] = w[k - 1, :k] @ A[i].T
        w[k, k] = c[i]
        cur = A[i] @ cur
        Phi[k] = cur
    T = Phi[C - 1]
    V = w[C - 1].T.copy()  # (9, C)

    Wm = np.zeros((C, 105), np.float64)   # X-coefficient block of lhsT
    U = np.zeros((9, 105), np.float64)    # sigma-coefficient block of lhsT
    for k in range(C):
        sel = [0, 1, 2 + slots[k]]
        U[:, 3 * k:3 * k + 3] = Phi[k][sel].T
        for j in range(k + 1):
            Wm[j, 3 * k:3 * k + 3] = w[k, j][sel]

    Tpow = [np.eye(9)]
    for _ in range(G + 1):
        Tpow.append(T @ Tpow[-1])

    ws1 = np.zeros((126, 126), np.float64)
    for j in range(G + 1):
        ws1[117:126, 9 * j:9 * j + 9] = Tpow[j].T
    wqv = np.zeros((G, C, 126), np.float64)
    for i in range(G):
        for j in range(i + 1, G + 1):
            wqv[i, :, 9 * j:9 * j + 9] = (Tpow[j - 1 - i] @ V).T

    winit = np.zeros((7, 126), np.float64)
    winit[0, 0] = 1.0
    winit[0, 1] = -1.0
    winit[1, 1] = 1.0
    winit[0, 117] = 1.0
    winit[0, 118] = -1.0
    winit[1, 118] = 1.0
    for j in range(P):
        winit[j, 119 + j] += 1.0
        winit[0, 119 + j] += -1.0

    Wm_hi, Wm_lo = _hi_lo(Wm)
    U_hi, U_lo = _hi_lo(U)
    ws1_hi, ws1_lo = _hi_lo(ws1)
    wq_hi, wq_lo = _hi_lo(wqv)

    bf = ml_dtypes.bfloat16
    wm1 = np.zeros((79, 105), bf)
    wm1[0:35] = Wm_hi
    wm1[70:79] = U_hi
    wm2 = np.zeros((88, 105), bf)
    wm2[0:35] = Wm_lo
    wm2[35:70] = Wm_hi
    wm2[70:79] = U_lo
    wm2[79:88] = U_hi
    wqb = np.zeros((G, 70, 126), bf)
    wqb[:, 0:35] = wq_lo
    wqb[:, 35:70] = wq_hi

    return dict(wm1=wm1, wm2=wm2, ws1h=ws1_hi, ws1l=ws1_lo,
                wqa=np.ascontiguousarray(wq_hi), wqb=wqb,
                winit=winit.astype(np.float32))


def build_bass(bl=BL):
    """Build the per-core Bass module (SPMD: same module, sharded inputs)."""
    import concourse.bacc as bacc
    import concourse.mybir as mybir
    from concourse.tile import TileContext

    BF = mybir.dt.bfloat16
    F32 = mybir.dt.float32
    nhalf = min(NHALF, bl)
    nh = (bl + nhalf - 1) // nhalf

    nc = bacc.Bacc(None, target_bir_lowering=False, debug=False)
    xhl = nc.declare_dram_parameter("xhl", [2, L, bl], BF, isOutput=False)
    x0 = nc.declare_dram_parameter("x0", [7, bl], F32, isOutput=False)
    wm1_d = nc.declare_dram_parameter("wm1", [79, 105], BF, isOutput=False)
    wm2_d = nc.declare_dram_parameter("wm2", [88, 105], BF, isOutput=False)
    ws1h_d = nc.declare_dram_parameter("ws1h", [126, 126], BF, isOutput=False)
    ws1l_d = nc.declare_dram_parameter("ws1l", [126, 126], BF, isOutput=False)
    wqa_d = nc.declare_dram_parameter("wqa", [G, C, 126], BF, isOutput=False)
    wqb_d = nc.declare_dram_parameter("wqb", [G, 2 * C, 126], BF, isOutput=False)
    winit_d = nc.declare_dram_parameter("winit", [7, 126], F32, isOutput=False)
    out_d = nc.declare_dram_parameter("out", [3 * L, bl], F32, isOutput=True)

    with TileContext(nc) as tc:
        with (
            tc.tile_pool(name="consts", bufs=1) as consts,
            tc.tile_pool(name="xpool", bufs=2 * G) as xpool,
            tc.tile_pool(name="spool", bufs=3) as spool,
            tc.tile_pool(name="tpool", bufs=2) as tpool,
            tc.tile_pool(name="ypool", bufs=4) as ypool,
            tc.tile_pool(name="ypsum", bufs=2, space="PSUM") as ypsum,
            tc.tile_pool(name="spsum", bufs=2, space="PSUM") as spsum,
        ):
            wm1 = consts.tile([79, 105], BF)
            nc.scalar.dma_start(out=wm1[:], in_=wm1_d[:])
            wm2 = consts.tile([88, 105], BF)
            nc.scalar.dma_start(out=wm2[:], in_=wm2_d[:])
            ws1h = consts.tile([126, 126], BF)
            nc.scalar.dma_start(out=ws1h[:], in_=ws1h_d[:])
            ws1l = consts.tile([126, 126], BF)
            nc.scalar.dma_start(out=ws1l[:], in_=ws1l_d[:])
            wqa = consts.tile([C, G * 126], BF)
            wqb = consts.tile([2 * C, G * 126], BF)
            for i in range(G):
                nc.scalar.dma_start(out=wqa[:, i * 126:(i + 1) * 126], in_=wqa_d[i])
                nc.scalar.dma_start(out=wqb[:, i * 126:(i + 1) * 126], in_=wqb_d[i])
            winit = consts.tile([7, 126], F32)
            nc.scalar.dma_start(out=winit[:], in_=winit_d[:])
            xinit = consts.tile([7, bl], F32)
            nc.scalar.dma_start(out=xinit[:], in_=x0[:])

            # --- init: y_0 rows and s_0 state (zeros elsewhere by construction)
            ip = spsum.tile([126, bl], F32, tag="sp")
            for h in range(nh):
                hs = slice(h * nhalf, (h + 1) * nhalf)
                nc.tensor.matmul(ip[:, hs], lhsT=winit[:], rhs=xinit[:, hs],
                                 start=True, stop=True)
            y0 = ypool.tile([3, bl], F32, tag="y0")
            nc.vector.tensor_copy(out=y0[:], in_=ip[0:3, :])
            nc.scalar.dma_start(out=out_d[0:3, :], in_=y0[:])

            def split_state(psum_tile):
                """psum (126, bl) f32 -> sbuf (126, 2*bl) bf16 [hi | lo]."""
                shl = spool.tile([126, 2 * bl], BF, tag="sprev")
                nc.vector.tensor_copy(out=shl[:, 0:bl], in_=psum_tile[:])
                res = tpool.tile([126, bl], F32, tag="res")
                nc.vector.tensor_sub(out=res[:], in0=psum_tile[:],
                                     in1=shl[:, 0:bl])
                nc.vector.tensor_copy(out=shl[:, bl:2 * bl], in_=res[:])
                return shl

            sprev = split_state(ip)

            for g_ in range(NG):
                xg = []
                for i in range(G):
                    t0 = 1 + C * (G * g_ + i)
                    xt = xpool.tile([88, bl], BF, tag="xg")
                    # rows 0..34 = x_hi, 35..69 = x_lo for steps t0..t0+34
                    nc.sync.dma_start(out=xt[0:C, :], in_=xhl[0, t0:t0 + C, :])
                    nc.scalar.dma_start(out=xt[C:2 * C, :],
                                        in_=xhl[1, t0:t0 + C, :])
                    xg.append(xt)

                # --- group scan: all 13 chunk-entry states + next group state
                sp = spsum.tile([126, bl], F32, tag="sp")
                for h in range(nh):
                    hs = slice(h * nhalf, (h + 1) * nhalf)
                    nc.tensor.matmul(sp[:, hs], lhsT=ws1h[:],
                                     rhs=sprev[:, h * nhalf:h * nhalf + nhalf],
                                     start=True, stop=False)
                    nc.tensor.matmul(sp[:, hs], lhsT=ws1l[:],
                                     rhs=sprev[:, h * nhalf:h * nhalf + nhalf],
                                     start=False, stop=False)
                    nc.tensor.matmul(sp[:, hs], lhsT=ws1h[:],
                                     rhs=sprev[:, bl + h * nhalf:bl + h * nhalf + nhalf],
                                     start=False, stop=False)
                    for i in range(G):
                        nc.tensor.matmul(sp[:, hs],
                                         lhsT=wqa[:, i * 126:(i + 1) * 126],
                                         rhs=xg[i][0:C, hs],
                                         start=False, stop=False)
                        nc.tensor.matmul(sp[:, hs],
                                         lhsT=wqb[:, i * 126:(i + 1) * 126],
                                         rhs=xg[i][0:2 * C, hs],
                                         start=False, stop=(i == G - 1))
                sprev_g = split_state(sp)
                # scatter sigma hi/lo into rows 70..78 / 79..87 of chunk tiles
                for i in range(G):
                    nc.sync.dma_start(out=xg[i][70:79, :],
                                      in_=sprev_g[9 * i:9 * i + 9, 0:bl])
                    nc.scalar.dma_start(out=xg[i][79:88, :],
                                        in_=sprev_g[9 * i:9 * i + 9, bl:2 * bl])

                # --- pass 2: per-chunk outputs
                for i in range(G):
                    r0 = 3 * (1 + C * (G * g_ + i))
                    yp = ypsum.tile([105, bl], F32, tag="yp")
                    for h in range(nh):
                        hs = slice(h * nhalf, (h + 1) * nhalf)
                        # hi.hi pass: K=79 covers [Xh; (Xl ignored); sh]
                        nc.tensor.matmul(yp[:, hs], lhsT=wm1[:],
                                         rhs=xg[i][0:79, hs],
                                         start=True, stop=False)
                    for h in range(nh):
                        hs = slice(h * nhalf, (h + 1) * nhalf)
                        # cross pass: K=88 over [Xh; Xl; sh; sl]
                        nc.tensor.matmul(yp[:, hs], lhsT=wm2[:],
                                         rhs=xg[i][0:88, hs],
                                         start=False, stop=True)
                    ysb = ypool.tile([105, bl], F32, tag="ysb")
                    if i % 2 == 0:
                        nc.vector.tensor_copy(out=ysb[:], in_=yp[:])
                        nc.sync.dma_start(out=out_d[r0:r0 + 105, :], in_=ysb[:])
                    else:
                        nc.scalar.copy(out=ysb[:], in_=yp[:])
                        nc.scalar.dma_start(out=out_d[r0:r0 + 105, :], in_=ysb[:])
                sprev = sprev_g
    nc.compile()
    return nc


def _prep_inputs(x, alpha, beta, gamma):
    import ml_dtypes
    bf = ml_dtypes.bfloat16
    xs = np.asarray(x, dtype=np.float32).reshape(B, L)
    coeffs = _build_coeffs(float(alpha), float(beta), float(gamma))
    in_maps = []
    for m in range(NCORES):
        xT_m = np.ascontiguousarray(xs[m * BL:(m + 1) * BL].T)  # (L, BL) f32
        hi = xT_m.astype(bf)
        lo = (xT_m - hi.astype(np.float32)).astype(bf)
        xhl = np.empty((2, L, BL), bf)
        xhl[0] = hi
        xhl[1] = lo
        x0 = np.ascontiguousarray(xT_m[0:7])                    # (7, BL) f32
        in_maps.append({"xhl": xhl, "x0": x0, **coeffs})
    return in_maps


LAST_RESULT = None  # BassKernelResults of the most recent kernel() call


def kernel(x, alpha, beta, gamma):
    global LAST_RESULT
    from concourse.bass_utils import run_bass_kernel_spmd

    nc = build_bass(BL)
    in_maps = _prep_inputs(x, alpha, beta, gamma)
    res = run_bass_kernel_spmd(nc, in_maps, core_ids=list(range(NCORES)))
    LAST_RESULT = res
    outs = [r["out"] for r in res.results]          # each (3L, BL) float32
    y = np.empty((B, L, 3), np.float32)
    for m in range(NCORES):
        y[m * BL:(m + 1) * BL] = outs[m].T.reshape(BL, L, 3)
    return y


# revision 14
# speedup vs baseline: 1.3389x; 1.3389x over previous
"""Holt-Winters exponential smoothing (level/trend/seasonal, P=7) on 8 Trainium2
NeuronCores.

Math: the per-row recurrence is linear in a 9-dim state
s = [level, trend, buf_0..buf_6]:  s_t = A_{t%7} s_{t-1} + c_{t%7} x_t.
Steps t=1..4095 are processed in 117 chunks of C=35 steps (35 % 7 == 0 so every
chunk sees the same slot pattern and shares one coefficient set), grouped into
9 groups of G=13 chunks.  Per chunk the outputs are one K=123 matmul
  Y_c (105,B) = lhsT.T @ [X_hi; X_lo; X_hi; sig_hi; sig_lo]
plus one K=117 matmul against the group state tile; the chunk-entry states
sigma_c come from a per-group prefix-scan matmul over the group's stacked
inputs (one K=105 matmul per chunk + 3 state matmuls per group).  The only
sequential dependency is the 9-link group chain.

Precision: inputs and coefficients are split hi/lo into bf16 pairs
(x = hi + lo, residual ~2^-18 relative).  Full precision needs the three
products Wh.Xh, Wh.Xl, Wl.Xh; the duplicated X_hi rows let all three run in a
single K-stacked bf16 matmul (1 cycle/row vs fp32's 4).  All products are
exact in fp32 PSUM accumulation; dropped Wl.Xl is ~2^-18 relative.

Sharding: pure data-parallel over the batch axis (1024 rows per core).
"""

import numpy as np

P = 7
C = 35            # chunk size (steps); 35 % 7 == 0
G = 13            # chunks per group
NG = 9            # groups; NG*G*C == L-1
L = 4096
B = 8192
NCORES = 8
BL = B // NCORES  # 1024 batch rows per core
NHALF = 512       # matmul moving-dim tile (fp32 PSUM bank limit)


def _sigmoid(z):
    return 1.0 / (1.0 + np.exp(-z))


def _step_mats(a, b, g):
    """A_i (9x9), c_i (9,) for seasonal slot i, float64."""
    A, c = [], []
    for i in range(P):
        col = 2 + i
        Ai = np.zeros((9, 9), np.float64)
        ci = np.zeros(9, np.float64)
        Ai[0, 0] = 1 - a
        Ai[0, 1] = 1 - a
        Ai[0, col] += -a
        Ai[1, 0] = -a * b
        Ai[1, 1] = 1 - a * b
        Ai[1, col] += -a * b
        for j in range(P):
            Ai[2 + j, 2 + j] = 1.0
        Ai[col, :] = 0.0
        Ai[col, 0] = -g * (1 - a)
        Ai[col, 1] = -g * (1 - a)
        Ai[col, col] = g * a + 1 - g
        ci[0] = a
        ci[1] = a * b
        ci[col] = g * (1 - a)
        A.append(Ai)
        c.append(ci)
    return A, c


def _hi_lo(x):
    import ml_dtypes
    hi = x.astype(np.float32).astype(ml_dtypes.bfloat16)
    lo = (x.astype(np.float32) - hi.astype(np.float32)).astype(ml_dtypes.bfloat16)
    return hi, lo


def _build_coeffs(alpha, beta, gamma):
    """Host-precomputed stationary matrices (float64 -> bf16 hi/lo splits).

    X-tile row layout (123 rows): [X_hi; X_lo; X_hi; sig_hi; sig_lo]
    lhsT layouts (K x M):
      wma (123, 105): [Wm_hi; Wm_hi; Wm_lo; U_hi; U_hi]   pass-2 main
      wmb (13, 117, 105): rows 9j..9j+8 = U_lo             pass-2 vs state tile
      ws1h/ws1l (126, 126): scan state-propagation lhsT, hi and lo
      wq  (13, 105, 126): per-chunk scan lhsT [wq_hi; wq_hi; wq_lo]
      winit (7, 126) f32: init matmul (y_0 rows 0..2, s_0 rows 117..125)
    """
    import ml_dtypes
    a, b, g = _sigmoid(alpha), _sigmoid(beta), _sigmoid(gamma)
    A, c = _step_mats(a, b, g)
    slots = [(1 + k) % P for k in range(C)]

    Phi = np.zeros((C, 9, 9), np.float64)
    w = np.zeros((C, C, 9), np.float64)
    cur = np.eye(9)
    for k in range(C):
        i = slots[k]
        if k > 0:
            w[k, :k] = w[k - 1, :k] @ A[i].T
        w[k, k] = c[i]
        cur = A[i] @ cur
        Phi[k] = cur
    T = Phi[C - 1]
    V = w[C - 1].T.copy()  # (9, C)

    Wm = np.zeros((C, 105), np.float64)   # X-coefficient block of lhsT
    U = np.zeros((9, 105), np.float64)    # sigma-coefficient block of lhsT
    for k in range(C):
        sel = [0, 1, 2 + slots[k]]
        U[:, 3 * k:3 * k + 3] = Phi[k][sel].T
        for j in range(k + 1):
            Wm[j, 3 * k:3 * k + 3] = w[k, j][sel]

    Tpow = [np.eye(9)]
    for _ in range(G + 1):
        Tpow.append(T @ Tpow[-1])

    ws1 = np.zeros((126, 126), np.float64)
    for j in range(G + 1):
        ws1[117:126, 9 * j:9 * j + 9] = Tpow[j].T
    wqv = np.zeros((G, C, 126), np.float64)
    for i in range(G):
        for j in range(i + 1, G + 1):
            wqv[i, :, 9 * j:9 * j + 9] = (Tpow[j - 1 - i] @ V).T

    winit = np.zeros((7, 126), np.float64)
    winit[0, 0] = 1.0
    winit[0, 1] = -1.0
    winit[1, 1] = 1.0
    winit[0, 117] = 1.0
    winit[0, 118] = -1.0
    winit[1, 118] = 1.0
    for j in range(P):
        winit[j, 119 + j] += 1.0
        winit[0, 119 + j] += -1.0

    Wm_hi, Wm_lo = _hi_lo(Wm)
    U_hi, U_lo = _hi_lo(U)
    ws1_hi, ws1_lo = _hi_lo(ws1)
    wq_hi, wq_lo = _hi_lo(wqv)

    bf = ml_dtypes.bfloat16
    wma = np.zeros((123, 105), bf)
    wma[0:35] = Wm_hi
    wma[35:70] = Wm_hi
    wma[70:105] = Wm_lo
    wma[105:114] = U_hi
    wma[114:123] = U_hi
    wmb = np.zeros((G, 117, 105), bf)
    for j in range(G):
        wmb[j, 9 * j:9 * j + 9] = U_lo
    wq = np.zeros((G, 105, 126), bf)
    wq[:, 0:35] = wq_hi
    wq[:, 35:70] = wq_hi
    wq[:, 70:105] = wq_lo

    return dict(wma=wma, wmb=wmb, ws1h=ws1_hi, ws1l=ws1_lo, wq=wq,
                winit=winit.astype(np.float32))


def build_bass(bl=BL):
    """Build the per-core Bass module (SPMD: same module, sharded inputs)."""
    import concourse.bacc as bacc
    import concourse.mybir as mybir
    from concourse.tile import TileContext

    BF = mybir.dt.bfloat16
    F32 = mybir.dt.float32
    nhalf = min(NHALF, bl)
    nh = (bl + nhalf - 1) // nhalf

    nc = bacc.Bacc(None, target_bir_lowering=False, debug=False)
    xhl = nc.declare_dram_parameter("xhl", [2, L, bl], BF, isOutput=False)
    x0 = nc.declare_dram_parameter("x0", [7, bl], F32, isOutput=False)
    wma_d = nc.declare_dram_parameter("wma", [123, 105], BF, isOutput=False)
    wmb_d = nc.declare_dram_parameter("wmb", [G, 117, 105], BF, isOutput=False)
    ws1h_d = nc.declare_dram_parameter("ws1h", [126, 126], BF, isOutput=False)
    ws1l_d = nc.declare_dram_parameter("ws1l", [126, 126], BF, isOutput=False)
    wq_d = nc.declare_dram_parameter("wq", [G, 105, 126], BF, isOutput=False)
    winit_d = nc.declare_dram_parameter("winit", [7, 126], F32, isOutput=False)
    out_d = nc.declare_dram_parameter("out", [3 * L, bl], F32, isOutput=True)

    with TileContext(nc) as tc:
        with (
            tc.tile_pool(name="consts", bufs=1) as consts,
            tc.tile_pool(name="xpool", bufs=2 * G) as xpool,
            tc.tile_pool(name="spool", bufs=3) as spool,
            tc.tile_pool(name="tpool", bufs=2) as tpool,
            tc.tile_pool(name="ypool", bufs=6) as ypool,
            tc.tile_pool(name="ypsum", bufs=2, space="PSUM") as ypsum,
            tc.tile_pool(name="spsum", bufs=2, space="PSUM") as spsum,
        ):
            wma = consts.tile([123, 105], BF)
            nc.scalar.dma_start(out=wma[:], in_=wma_d[:])
            wmb = consts.tile([117, G * 105], BF)
            ws1h = consts.tile([126, 126], BF)
            nc.scalar.dma_start(out=ws1h[:], in_=ws1h_d[:])
            ws1l = consts.tile([126, 126], BF)
            nc.scalar.dma_start(out=ws1l[:], in_=ws1l_d[:])
            wq = consts.tile([105, G * 126], BF)
            for i in range(G):
                nc.scalar.dma_start(out=wq[:, i * 126:(i + 1) * 126], in_=wq_d[i])
                nc.scalar.dma_start(out=wmb[:, i * 105:(i + 1) * 105], in_=wmb_d[i])
            winit = consts.tile([7, 126], F32)
            nc.scalar.dma_start(out=winit[:], in_=winit_d[:])
            xinit = consts.tile([7, bl], F32)
            nc.scalar.dma_start(out=xinit[:], in_=x0[:])

            # --- init: y_0 rows and s_0 state (zeros elsewhere by construction)
            ip = spsum.tile([126, bl], F32, tag="sp")
            for h in range(nh):
                hs = slice(h * nhalf, (h + 1) * nhalf)
                nc.tensor.matmul(ip[:, hs], lhsT=winit[:], rhs=xinit[:, hs],
                                 start=True, stop=True)
            y0 = ypool.tile([3, bl], F32, tag="y0")
            nc.vector.tensor_copy(out=y0[:], in_=ip[0:3, :])
            nc.scalar.dma_start(out=out_d[0:3, :], in_=y0[:])

            def split_state(psum_tile):
                """psum (126, bl) f32 -> sbuf (126, 2*bl) bf16 [hi | lo]."""
                shl = spool.tile([126, 2 * bl], BF, tag="sprev")
                nc.vector.tensor_copy(out=shl[:, 0:bl], in_=psum_tile[:])
                res = tpool.tile([126, bl], F32, tag="res")
                nc.vector.tensor_sub(out=res[:], in0=psum_tile[:],
                                     in1=shl[:, 0:bl])
                nc.vector.tensor_copy(out=shl[:, bl:2 * bl], in_=res[:])
                return shl

            sprev = split_state(ip)

            for g_ in range(NG):
                xg = []
                for i in range(G):
                    t0 = 1 + C * (G * g_ + i)
                    xt = xpool.tile([123, bl], BF, tag="xg")
                    nc.scalar.dma_start(out=xt[0:C, :], in_=xhl[0, t0:t0 + C, :])
                    nc.scalar.dma_start(out=xt[C:2 * C, :],
                                        in_=xhl[1, t0:t0 + C, :])
                    # duplicate X_hi locally (SBUF->SBUF, off the HBM path)
                    nc.sync.dma_start(out=xt[2 * C:3 * C, :], in_=xt[0:C, :])
                    xg.append(xt)

                # --- group scan: all 13 chunk-entry states + next group state
                sp = spsum.tile([126, bl], F32, tag="sp")
                for h in range(nh):
                    hs = slice(h * nhalf, (h + 1) * nhalf)
                    nc.tensor.matmul(sp[:, hs], lhsT=ws1h[:],
                                     rhs=sprev[:, h * nhalf:h * nhalf + nhalf],
                                     start=True, stop=False)
                    nc.tensor.matmul(sp[:, hs], lhsT=ws1h[:],
                                     rhs=sprev[:, bl + h * nhalf:bl + h * nhalf + nhalf],
                                     start=False, stop=False)
                    nc.tensor.matmul(sp[:, hs], lhsT=ws1l[:],
                                     rhs=sprev[:, h * nhalf:h * nhalf + nhalf],
                                     start=False, stop=False)
                    for i in range(G):
                        nc.tensor.matmul(sp[:, hs],
                                         lhsT=wq[:, i * 126:(i + 1) * 126],
                                         rhs=xg[i][0:105, hs],
                                         start=False, stop=(i == G - 1))
                sprev_g = split_state(sp)
                # scatter sigma hi/lo into rows 105..113 / 114..122
                for i in range(G):
                    nc.sync.dma_start(out=xg[i][105:114, :],
                                      in_=sprev_g[9 * i:9 * i + 9, 0:bl])
                    nc.sync.dma_start(out=xg[i][114:123, :],
                                      in_=sprev_g[9 * i:9 * i + 9, bl:2 * bl])

                # --- pass 2: per-chunk outputs
                for i in range(G):
                    r0 = 3 * (1 + C * (G * g_ + i))
                    yp = ypsum.tile([105, bl], F32, tag="yp")
                    for h in range(nh):
                        hs = slice(h * nhalf, (h + 1) * nhalf)
                        nc.tensor.matmul(yp[:, hs], lhsT=wma[:],
                                         rhs=xg[i][0:123, hs],
                                         start=True, stop=False)
                    for h in range(nh):
                        hs = slice(h * nhalf, (h + 1) * nhalf)
                        nc.tensor.matmul(yp[:, hs],
                                         lhsT=wmb[:, i * 105:(i + 1) * 105],
                                         rhs=sprev_g[0:117, h * nhalf:h * nhalf + nhalf],
                                         start=False, stop=True)
                    ysb = ypool.tile([105, bl], F32, tag="ysb")
                    nc.vector.tensor_copy(out=ysb[:], in_=yp[:])
                    if i % 2 == 0:
                        nc.sync.dma_start(out=out_d[r0:r0 + 105, :], in_=ysb[:])
                    else:
                        nc.scalar.dma_start(out=out_d[r0:r0 + 105, :], in_=ysb[:])
                sprev = sprev_g
    nc.compile()
    return nc


def _prep_inputs(x, alpha, beta, gamma):
    import ml_dtypes
    bf = ml_dtypes.bfloat16
    xs = np.asarray(x, dtype=np.float32).reshape(B, L)
    coeffs = _build_coeffs(float(alpha), float(beta), float(gamma))
    in_maps = []
    for m in range(NCORES):
        xT_m = np.ascontiguousarray(xs[m * BL:(m + 1) * BL].T)  # (L, BL) f32
        hi = xT_m.astype(bf)
        lo = (xT_m - hi.astype(np.float32)).astype(bf)
        xhl = np.empty((2, L, BL), bf)
        xhl[0] = hi
        xhl[1] = lo
        x0 = np.ascontiguousarray(xT_m[0:7])                    # (7, BL) f32
        in_maps.append({"xhl": xhl, "x0": x0, **coeffs})
    return in_maps


LAST_RESULT = None  # BassKernelResults of the most recent kernel() call


def kernel(x, alpha, beta, gamma):
    global LAST_RESULT
    from concourse.bass_utils import run_bass_kernel_spmd

    nc = build_bass(BL)
    in_maps = _prep_inputs(x, alpha, beta, gamma)
    res = run_bass_kernel_spmd(nc, in_maps, core_ids=list(range(NCORES)))
    LAST_RESULT = res
    outs = [r["out"] for r in res.results]          # each (3L, BL) float32
    y = np.empty((B, L, 3), np.float32)
    for m in range(NCORES):
        y[m * BL:(m + 1) * BL] = outs[m].T.reshape(BL, L, 3)
    return y
